# revision 37
# baseline (speedup 1.0000x reference)
"""Multi-head attention (B=2, S=2048, D=1024, H=16) on 8 TRN2 NeuronCores.

Sharding: tensor-parallel on heads (2 heads = 128 channels per core).
Everything on-device runs in "transposed" layout [channel, B*S]:
  - host passes hT pre-arranged so each tensor loads with ONE contiguous
    DMA (hT in 8 column blocks); hT stays resident in SBUF (64KB/part)
  - per-core Q/K/V projections produce qT/kT/vT [128, B*S]
  - attention per (batch, 512-query-block) with BOTH heads packed into
    one [128, 1024] score PSUM tile (h0 cols 0-511, h1 cols 512-1023):
      ONE exp covers both heads (mask bias is per key-partition, shared);
      per-head PV accumulates into [65, 512] PSUM (ones-row = denom);
      QK runs one key tile AHEAD of PV so the ACT exp stream never waits
      on the exp->PV->QK semaphore chain (ACT is the attention floor at
      ~1.11us per packed exp)
  - normalization: sums land at partitions 0/64 during ctx evacuation,
    sel matmul broadcasts them, reciprocal + multiply (on GpSimd);
    emitted as a filler pop inside the NEXT block so its sel matmul
    never stalls the PE stream at a block boundary
  - output projection: partial sums outT[o, n] += Wo[o, own 128 chans]
    @ ctxn, written bf16; host reduces across cores and adds bo. PSUM
    evacuation alternates DVE/ACT so neither engine paces the chain.

Phase emission order keeps the TensorE stream dense: 96 identity
warmup matmuls bridge the DMA-bound head (keeping HAM/p-state at full
clock), the first attention block starts after only chunk 0's k/q
projections (the rest fed just-in-time as hT DMA blocks land on three
DGE queues), and projections for the other batch plus output
projection for finished blocks are popped as "filler" inside the
ScalarE-bound attention inner loop.

PSUM budget (8 banks): sc 2x[128,1024] (4) + ctx 2x[65,512] (2) +
pj 1x[128,512] (1) + po 1x[128,512] (1).
"""

import numpy as np
import ml_dtypes

import concourse.bass as bass
import concourse.mybir as mybir
import concourse.tile as tile
from concourse import bacc
from concourse import bass_utils

F32 = mybir.dt.float32
BF16 = mybir.dt.bfloat16
BF16_NP = ml_dtypes.bfloat16

B, S, D, H = 2, 2048, 1024, 16
HD = D // H
BS = B * S            # 4096
P = 128               # partitions / channels per core
NCORES = 8
KT = S // P           # 16 key tiles per batch
NQ = 512              # matmul moving free dim
VA_W = HD + 1         # v_aug columns per key tile (64 v cols + ones col)
QB = 512              # attention query block (ctx PSUM bank width)
NB = S // QB          # 4 query blocks per batch

_CACHE = {}


def _build():
    nc = bacc.Bacc("TRN2", target_bir_lowering=False, debug=False,
                   num_devices=NCORES)

    # all host-side pre-arranged for single contiguous DMAs
    hT = nc.dram_tensor("hT", [P, (D // P) * BS], BF16, kind="ExternalInput")
    wq = nc.dram_tensor("wq", [P, D], BF16, kind="ExternalInput")
    wk = nc.dram_tensor("wk", [P, D], BF16, kind="ExternalInput")
    wv = nc.dram_tensor("wv", [P, D], BF16, kind="ExternalInput")
    wo = nc.dram_tensor("wo", [P, D], BF16, kind="ExternalInput")
    bq = nc.dram_tensor("bq", [P, 1], F32, kind="ExternalInput")
    bk = nc.dram_tensor("bk", [P, 1], F32, kind="ExternalInput")
    bv = nc.dram_tensor("bv", [P, 1], F32, kind="ExternalInput")
    maskP = nc.dram_tensor("maskP", [P, B * KT], F32, kind="ExternalInput")
    sel = nc.dram_tensor("sel", [HD + 1, P], BF16, kind="ExternalInput")
    idn = nc.dram_tensor("idn", [P, P], BF16, kind="ExternalInput")
    outT = nc.dram_tensor("outT", [D, BS], BF16, kind="ExternalOutput")

    with tile.TileContext(nc) as tc:
        with (
            tc.tile_pool(name="const", bufs=1) as const,
            tc.tile_pool(name="res", bufs=1) as res,
            tc.tile_pool(name="va", bufs=2) as va_pool,
            tc.tile_pool(name="pr", bufs=3) as pr_pool,
            tc.tile_pool(name="bc", bufs=2) as bc_pool,
            tc.tile_pool(name="ot", bufs=4) as ot_pool,
            tc.tile_pool(name="pj_ps", bufs=1, space="PSUM") as pj_ps,
            tc.tile_pool(name="po_ps", bufs=1, space="PSUM") as po_ps,
            tc.tile_pool(name="sc_ps", bufs=2, space="PSUM") as sc_ps,
            tc.tile_pool(name="ctx_ps", bufs=2, space="PSUM") as ctx_ps,
        ):
            # ---- constants / weights in SBUF (contiguous DMAs) ----
            # The 8.75MB input load is HBM-bound per queue (~165GB/s
            # observed), so spread it over all three DGE queues (SP +
            # Activation HWDGE, GpSimd SWDGE). hT block k is needed at
            # roughly (7 + 3k)us; weights gate the very first matmul so
            # they lead the scalar queue.
            # one SBUF tile PER hT block: Tile's dependency tracking is
            # conservative per-tile (any read waits ALL writes), so a
            # single big tile pinned the first projection to the LAST
            # block's DMA (~35us) instead of its own block's (~17us)
            BLK = (D // P) * NQ   # 4096 cols per column block
            ht_blks = [const.tile([P, BLK], BF16, name=f"htb{n}",
                                  tag=f"htb{n}") for n in range(B * NB)]
            w_sbs = {}
            b_sbs = {}

            def _hblk(eng, n):
                eng.dma_start(ht_blks[n][:], hT.ap()[:, n * BLK:(n + 1) * BLK])

            for nm, w in (("wk", wk), ("wq", wq), ("wv", wv)):
                t = const.tile([P, D], BF16, name=f"{nm}_sb", tag=f"{nm}_sb")
                nc.scalar.dma_start(t[:], w.ap())
                w_sbs[nm] = t
            for nm, bt in (("bq", bq), ("bk", bk), ("bv", bv)):
                t = const.tile([P, 1], F32, name=f"{nm}_sb", tag=f"{nm}_sb")
                nc.scalar.dma_start(t[:], bt.ap())
                b_sbs[nm] = t
            identF = const.tile([P, P], BF16)
            nc.sync.dma_start(identF[:], idn.ap())
            _hblk(nc.sync, 0)
            mask_sb = const.tile([P, B * KT], F32)
            nc.gpsimd.dma_start(mask_sb[:], maskP.ap())
            # batch-0 blocks lead the two HWDGE queues (sync/scalar);
            # the gpsimd SWDGE queue barely moves before ~28us, so it only
            # carries b1 blocks that attention needs late anyway.
            # DMA engines round-robin descriptors across a queue's
            # pending entries, so a block completes near its queue's total
            # drain — keep the attention-gating b0 blocks on queues with
            # minimal co-pending bytes.
            _hblk(nc.sync, 2)
            _hblk(nc.scalar, 1)
            _hblk(nc.scalar, 3)
            _hblk(nc.gpsimd, 4)
            _hblk(nc.gpsimd, 5)
            _hblk(nc.gpsimd, 6)
            _hblk(nc.gpsimd, 7)
            # PE warmup: the input load is DMA-bound for ~30us and an idle
            # PE sits in a low HAM/p-state gear, making the first real
            # matmuls 2-3x slow. Identity matmuls (no DMA deps) keep the PE
            # busy from ~6.5us so it is at full clock when data lands.
            warm_ps = po_ps.tile([P, P], F32, name="warm", tag="po")
            for _ in range(96):
                nc.tensor.matmul(warm_ps[:], identF[:], identF[:],
                                 start=True, stop=True)

            qT = res.tile([P, BS], BF16)
            kT = res.tile([P, BS], BF16)
            vT = res.tile([P, BS], BF16)
            ctxraw = res.tile([P, BS], F32)
            ctxn = res.tile([P, BS], BF16)
            # softmax sums: the ctx evacuation writes h0's ones-row to
            # partition 0 and h1's to partition 64 (DVE cross-partition
            # copies are only legal at multiple-of-64 offsets), so the sel
            # matmul consumes them with no relocation step. Rows 1-63 are
            # memset once and zeroed by sel's 0 coefficients.
            s2_sb = res.tile([HD + 1, BS], BF16)
            nc.vector.memset(s2_sb[:], 0.0)

            VA = {}

            def setup_va(b):
                vas = []
                for h in range(2):
                    va = va_pool.tile([P, KT * VA_W], BF16, name=f"va{b}{h}",
                                      tag=f"va{h}")
                    nc.vector.memset(va[:], 1.0)
                    vas.append(va)
                VA[b] = vas

            def one_proj(wn, bn, dest, n):
                """one projection for one 512-col chunk, yielding after
                every contraction matmul (~0.4us PE each) so filler pops
                stay inside the attention loop's per-kt PE slack."""
                ps = pj_ps.tile([P, NQ], F32, name=f"ps_{wn}", tag="pj")
                for k in range(D // P):
                    nc.tensor.matmul(
                        ps[:], w_sbs[wn][:, bass.ts(k, P)],
                        ht_blks[n][:, k * NQ:(k + 1) * NQ],
                        start=(k == 0), stop=(k == D // P - 1))
                    if k % 2 == 1:
                        yield
                # high_priority: this evac frees the single pj bank; a lazy
                # DVE turnaround here stalls every later filler matmul
                with tc.high_priority():
                    nc.vector.tensor_scalar_add(
                        dest[:, bass.ts(n, NQ)], ps[:], b_sbs[bn][:])
                yield

            def tr_steps(b, ktlo, kthi):
                """v transposes for key tiles [ktlo,kthi): one [128,128] PE
                transpose covers BOTH heads' v slices; alternate pj/po
                banks so the DVE evacuation never head-of-line-blocks."""
                vas = VA[b]
                boff = b * S
                for kt in range(ktlo, kthi):
                    pool = pj_ps if kt % 2 == 0 else po_ps
                    tp = pool.tile([P, P], BF16, name="tp",
                                   tag="pj" if kt % 2 == 0 else "po")
                    nc.tensor.transpose(
                        tp[:], vT[:, boff + kt * P:boff + (kt + 1) * P],
                        identF[:])
                    nc.vector.tensor_copy(
                        vas[0][:, kt * VA_W:kt * VA_W + HD], tp[:, 0:HD])
                    nc.vector.tensor_copy(
                        vas[1][:, kt * VA_W:kt * VA_W + HD], tp[:, HD:P])
                    yield

            def proj_va_steps(b, nlo, nhi, with_q=True):
                """K/V projections + v_aug build for column chunks
                [nlo,nhi) of batch b (PE filler inside attention). k first:
                attention QKs gate on kT."""
                for n in range(b * NB + nlo, b * NB + nhi):
                    yield from one_proj("wk", "bk", kT, n)
                    yield from one_proj("wv", "bv", vT, n)
                    if with_q:
                        yield from one_proj("wq", "bq", qT, n)
                    nlocal = n - b * NB
                    yield from tr_steps(b, nlocal * 4, nlocal * 4 + 4)

            def chain(*gens):
                for g in gens:
                    yield from g

            def qk_pair(b, qb, kt):
                boff = b * S
                qsl = slice(boff + qb * QB, boff + (qb + 1) * QB)
                ksl = slice(boff + kt * P, boff + (kt + 1) * P)
                sct = sc_ps.tile([P, 2 * QB], F32, name="sct", tag="sct")
                nc.tensor.matmul(sct[:, 0:QB], kT[0:HD, ksl],
                                 qT[0:HD, qsl], start=True, stop=True)
                nc.tensor.matmul(sct[:, QB:2 * QB], kT[HD:P, ksl],
                                 qT[HD:P, qsl], start=True, stop=True)
                return sct

            def attn_qb(b, qb, filler, pops=1, quiet_head=0, quiet_tail=0,
                        sct0=None, next_blk=None):
                """Both heads' attention for one 512-query block. Each key
                tile: two row-group QK matmuls into one packed [128,1024]
                score tile, one exp for both heads, two PV accumulations.
                Pops filler steps to keep the PE dense. The NEXT block's
                first QK is emitted inside this block's last iteration
                (cross-block pipelining) and its sct handed forward, so the
                next block's first exp never waits behind PV(15)."""
                va0, va1 = VA[b]
                ctx0 = ctx_ps.tile([VA_W, QB], F32, name="ctx0", tag="ctx")
                ctx1 = ctx_ps.tile([VA_W, QB], F32, name="ctx1", tag="ctx")

                # QK runs one key tile AHEAD of PV: the per-kt PE stream is
                # [QK(kt+1), pops, PV(kt)], so exp(kt+1) never waits on the
                # exp(kt)->PV(kt)->QK(kt+1) semaphore chain — the ACT
                # stream stays saturated at its 1.11us/exp floor.
                sct = sct0 if sct0 is not None else qk_pair(b, qb, 0)
                nxt = None
                for kt in range(KT):
                    pr = pr_pool.tile([P, 2 * QB], BF16, name="pr", tag="pr")
                    nc.scalar.activation(
                        pr[:], sct[:], mybir.ActivationFunctionType.Exp,
                        bias=mask_sb[:, b * KT + kt:b * KT + kt + 1],
                        scale=0.125)
                    if kt + 1 < KT:
                        sct = qk_pair(b, qb, kt + 1)
                    elif next_blk is not None:
                        nxt = qk_pair(next_blk[0], next_blk[1], 0)
                    # pops sit BETWEEN the QK pair and PV: a DMA- or
                    # DVE-gated filler here overlaps the exp wait instead of
                    # head-of-line-blocking the next block's QKs
                    if filler is not None and \
                            quiet_head <= kt < KT - quiet_tail:
                        for _ in range(pops):
                            next(filler, None)
                    nc.tensor.matmul(
                        ctx0[:], va0[:, kt * VA_W:(kt + 1) * VA_W],
                        pr[:, 0:QB], start=(kt == 0), stop=(kt == KT - 1))
                    nc.tensor.matmul(
                        ctx1[:], va1[:, kt * VA_W:(kt + 1) * VA_W],
                        pr[:, QB:2 * QB], start=(kt == 0),
                        stop=(kt == KT - 1))
                # fast evacuation: plain DVE copies release the ctx PSUM
                # slots; reciprocal happens later off-PSUM. high_priority so
                # the next block's PV reuse isn't blocked on a lazy DVE.
                qsl = slice(b * S + qb * QB, b * S + (qb + 1) * QB)
                with tc.high_priority():
                    nc.vector.tensor_copy(ctxraw[0:HD, qsl], ctx0[0:HD, :])
                    nc.vector.tensor_copy(s2_sb[0:1, qsl],
                                          ctx0[HD:HD + 1, :])
                    nc.vector.tensor_copy(ctxraw[HD:P, qsl], ctx1[0:HD, :])
                    nc.vector.tensor_copy(s2_sb[HD:HD + 1, qsl],
                                          ctx1[HD:HD + 1, :])
                return nxt

            def gather_norm(b, qb, use_dve=False):
                """normalize ctxT for one 512-column block: broadcast the
                two heads' sums (at partitions 0/64) via sel matmul, recip,
                multiply. high_priority so the chain threads in. The
                multiply runs on GpSimd (idle engine, SBUF-only operands)
                to keep the DVE free — except use_dve for the final block,
                where DVE's lower dispatch latency shortens the tail."""
                with tc.high_priority():
                    goff = b * S + qb * QB
                    pbc = po_ps.tile([P, QB], F32, name="pbc", tag="po")
                    nc.tensor.matmul(pbc[:], sel_sb[:],
                                     s2_sb[:, goff:goff + QB],
                                     start=True, stop=True)
                    bcr = bc_pool.tile([P, QB], F32, name="bcr", tag="bcr")
                    nc.vector.reciprocal_approx_fast(bcr[:], pbc[:])
                    eng = nc.vector if use_dve else nc.gpsimd
                    eng.tensor_mul(
                        ctxn[:, goff:goff + QB],
                        ctxraw[:, goff:goff + QB], bcr[:])

            def gn_gen(b, qb):
                """gather_norm as a single-pop filler: emitted inside the
                NEXT block's kt loop, so its sel matmul never sits in the
                PE stream at a block boundary waiting on the sums evac."""
                gather_norm(b, qb)
                yield

            def oproj_steps(b, blo, bhi, tail=False):
                """partial output projection for 512-col blocks [blo,bhi) of
                batch b: outT[o, n] += Wo[o, own chans] @ ctxn — full o
                range, own 128 channels; cross-core reduction on host.
                PSUM evacuation alternates DVE/ACT: a single engine paces
                the whole chain at ~0.9us per matmul (bank round-trip) and
                that crawl head-of-line-blocks the attention QKs behind it.
                tail mode (attention done, ctx banks free) rotates 4 PSUM
                banks and 2 DMA queues to minimize the drain latency."""
                boff = b * S
                pools = ([po_ps, pj_ps, ctx_ps, ctx_ps] if tail
                         else [po_ps, pj_ps])
                tags = ["po", "pj", "ctx", "ctx"]
                for cg in range(blo, bhi):
                    goff = boff + cg * QB
                    for t in range(D // P):
                        m = t % len(pools)
                        po = pools[m].tile([P, QB], F32, name="po",
                                           tag=tags[m])
                        nc.tensor.matmul(
                            po[:], wo_sb[:, bass.ts(t, P)],
                            ctxn[:, goff:goff + QB],
                            start=True, stop=True)
                        ot = ot_pool.tile([P, QB], BF16, name="ot", tag="ot")
                        if t % 2 == 1:
                            nc.scalar.activation(
                                ot[:], po[:],
                                mybir.ActivationFunctionType.Copy, bias=0.0)
                        else:
                            nc.vector.tensor_copy(ot[:], po[:])
                        if tail:
                            dq = nc.scalar if t % 2 == 1 else nc.gpsimd
                        else:
                            dq = nc.sync
                        dq.dma_start(
                            outT.ap()[bass.ts(t, P), goff:goff + QB], ot[:])
                        if t % 2 == 1:
                            yield
                    yield

            class FQ:
                """Filler queue: generators become poppable only once
                pushed, so a filler that reads a region (e.g. o-proj on
                ctxn) is never EMITTED before its producer (gather_norm)
                — Tile deps are emission-order-based."""

                def __init__(self):
                    self.gens = []

                def push(self, g):
                    self.gens.append(g)

                def push_front(self, g):
                    self.gens.insert(0, g)

                def __next__(self):
                    while self.gens:
                        try:
                            return next(self.gens[0])
                        except StopIteration:
                            self.gens.pop(0)
                    return None

            def drain(g):
                if isinstance(g, FQ):
                    while g.gens:
                        next(g)
                    return
                for _ in g:
                    pass

            # software pipeline: engines run their streams in-order, so
            # anything that waits on a slow dependency must sit at a stream
            # position where that dependency is already resolved.
            setup_va(0)
            # only chunk 0's k and q projected serially; v+transposes and
            # chunks 1-3 are filler inside the first attention block
            # (QK(kt) gates on kT chunk kt//4, PV(kt) on va(kt) — produced
            # just in time as the DMA blocks land). Attention's exp stream
            # starts as soon as the first hT block is projected.
            drain(chain(one_proj("wk", "bk", kT, 0),
                        one_proj("wq", "bq", qT, 0)))
            # deferred constant loads: not needed until gather_norm/o-proj,
            # so keep them out of the startup DMA burst
            wo_sb = const.tile([P, D], BF16)
            nc.sync.dma_start(wo_sb[:], wo.ap())
            sel_sb = const.tile([HD + 1, P], BF16)
            nc.sync.dma_start(sel_sb[:], sel.ap())
            # pops=6: fA0's yield c for PV(kt)/QK(kt) emission deadlines —
            # v-ch0 y1-5, tr kt0-3 y6-9, chunk c at y10+19(c-1): PV(kt)
            # sees (kt+1)*p pops, QK(kt) sees kt*p; p=6 satisfies all
            # (tightest: PV(0) needs y6 <= 6).
            fA0 = chain(one_proj("wv", "bv", vT, 0), tr_steps(0, 0, 4),
                        proj_va_steps(0, 1, 4))
            s = attn_qb(0, 0, fA0, pops=6, next_blk=(0, 1))
            drain(fA0)
            setup_va(1)
            # b1 k/v proj as filler in A(b0); ALL of b1's q proj is
            # deferred into A(b1) to balance the two windows' PE load.
            # quiet_head on qb1: b1's hT blocks are still in flight; a
            # DMA-gated filler matmul would head-of-line-block the QKs
            # behind it in the PE stream.
            fq0 = FQ()
            fq0.push(gn_gen(0, 0))
            fq0.push(proj_va_steps(1, 0, 4, with_q=False))
            s = attn_qb(0, 1, fq0, pops=2, quiet_head=6, sct0=s,
                        next_blk=(0, 2))
            fq0.push_front(gn_gen(0, 1))
            s = attn_qb(0, 2, fq0, pops=2, sct0=s, next_blk=(0, 3))
            fq0.push_front(gn_gen(0, 2))
            # q0 (b1 chunk-0 q proj) is popped inside attn(0,3) so the
            # cross-block QK(1,0,kt0) emitted at its tail has its input
            fq0.push(one_proj("wq", "bq", qT, NB))
            s = attn_qb(0, 3, fq0, pops=2, quiet_tail=2, sct0=s,
                        next_blk=(1, 0))
            fq0.push_front(gn_gen(0, 3))
            drain(fq0)                          # b1 k/v/q0 + b0 norms done
            fq = FQ()                           # A(b1) fillers
            q1 = one_proj("wq", "bq", qT, NB + 1)
            fq.push(q1)
            fq.push(oproj_steps(0, 0, NB))
            s = attn_qb(1, 0, fq, pops=2, quiet_head=2, quiet_tail=3,
                        sct0=s, next_blk=(1, 1))
            drain(q1)                           # qT chunk 1 for attn(1,1)
            fq.push_front(gn_gen(1, 0))
            fq.push(oproj_steps(1, 0, 1))
            q2 = one_proj("wq", "bq", qT, NB + 2)
            fq.push(q2)
            s = attn_qb(1, 1, fq, pops=2, quiet_tail=3, sct0=s,
                        next_blk=(1, 2))
            drain(q2)
            fq.push_front(gn_gen(1, 1))
            fq.push(oproj_steps(1, 1, 2))
            q3 = one_proj("wq", "bq", qT, NB + 3)
            fq.push(q3)
            s = attn_qb(1, 2, fq, pops=2, quiet_tail=3, sct0=s,
                        next_blk=(1, 3))
            drain(q3)
            fq.push_front(gn_gen(1, 2))
            fq.push(oproj_steps(1, 2, 3))
            attn_qb(1, 3, fq, pops=2, quiet_tail=3, sct0=s)
            drain(fq)
            gather_norm(1, 3, use_dve=True)
            drain(oproj_steps(1, NB - 1, NB, tail=True))

    nc.compile()
    return nc


def _prep_inputs(hidden_state, attention_mask, Wq, bq, Wk, bk, Wv, bv, Wo,
                 bo):
    # hT blocks: hTr[p, (n*8+k)*512 + m] = h2[k*128+p, n*512+m]
    h2 = np.ascontiguousarray(
        np.asarray(hidden_state, dtype=np.float32).reshape(BS, D).T)
    h3 = h2.reshape(D // P, P, B * NB, NQ)          # [k, p, n, m]
    hTr = np.ascontiguousarray(
        h3.transpose(1, 2, 0, 3).reshape(P, (D // P) * BS)).astype(BF16_NP)
    # mask: maskP[p, b*KT + t] = mask[b, t*128+p]
    m2 = np.asarray(attention_mask, dtype=np.float32).reshape(B, S)
    maskP = np.ascontiguousarray(
        m2.reshape(B, KT, P).transpose(2, 0, 1).reshape(P, B * KT))
    idnm = np.eye(P, dtype=np.float32).astype(BF16_NP)
    selm = np.zeros((HD + 1, P), dtype=BF16_NP)
    selm[0, 0:HD] = 1
    selm[HD, HD:P] = 1

    def warr(Wslice):
        # w[p, k*128 + c] = Wslice.T[k*128+p, c]
        wt = np.asarray(Wslice, dtype=np.float32).T     # [D, P]
        return np.ascontiguousarray(
            wt.reshape(D // P, P, P).transpose(1, 0, 2).reshape(P, D)
        ).astype(BF16_NP)

    in_maps = []
    for c in range(NCORES):
        sl = slice(c * P, (c + 1) * P)
        in_maps.append({
            "hT": hTr,
            "wq": warr(np.asarray(Wq)[sl, :]),
            "wk": warr(np.asarray(Wk)[sl, :]),
            "wv": warr(np.asarray(Wv)[sl, :]),
            "wo": np.ascontiguousarray(
                np.asarray(Wo, dtype=np.float32)[:, sl].T).astype(BF16_NP),
            "bq": np.asarray(bq, dtype=np.float32)[sl].reshape(P, 1),
            "bk": np.asarray(bk, dtype=np.float32)[sl].reshape(P, 1),
            "bv": np.asarray(bv, dtype=np.float32)[sl].reshape(P, 1),
            "maskP": maskP,
            "sel": selm,
            "idn": idnm,
        })
    return in_maps


def kernel(**inputs) -> np.ndarray:
    if "nc" not in _CACHE:
        _CACHE["nc"] = _build()
    nc = _CACHE["nc"]
    in_maps = _prep_inputs(**inputs)
    res = bass_utils.run_bass_kernel_spmd(
        nc, in_maps, core_ids=list(range(NCORES)))
    outT = res.results[0]["outT"].astype(np.float32)  # [D, BS] partials
    for c in range(1, NCORES):
        outT += res.results[c]["outT"].astype(np.float32)
    out = np.ascontiguousarray(outT.T).reshape(B, S, D)
    out += np.asarray(inputs["bo"], dtype=np.float32)
    return out.astype(np.float32)


# revision 39
# speedup vs baseline: 1.0102x; 1.0102x over previous
"""Multi-head attention (B=2, S=2048, D=1024, H=16) on 8 TRN2 NeuronCores.

Sharding: tensor-parallel on heads (2 heads = 128 channels per core).
Everything on-device runs in "transposed" layout [channel, B*S]:
  - host passes hT pre-arranged so each tensor loads with ONE contiguous
    DMA (hT in 8 column blocks); hT stays resident in SBUF (64KB/part)
  - per-core Q/K/V projections produce qT/kT/vT [128, B*S]
  - attention per (batch, 512-query-block) with BOTH heads packed into
    one [128, 1024] score PSUM tile (h0 cols 0-511, h1 cols 512-1023):
      ONE exp covers both heads (mask bias is per key-partition, shared);
      per-head PV accumulates into [65, 512] PSUM (ones-row = denom);
      QK runs one key tile AHEAD of PV so the ACT exp stream never waits
      on the exp->PV->QK semaphore chain (ACT is the attention floor at
      ~1.11us per packed exp)
  - normalization: sums land at partitions 0/64 during ctx evacuation,
    sel matmul broadcasts them, reciprocal + multiply (on GpSimd);
    emitted as a filler pop inside the NEXT block so its sel matmul
    never stalls the PE stream at a block boundary
  - output projection: partial sums outT[o, n] += Wo[o, own 128 chans]
    @ ctxn, written bf16; host reduces across cores and adds bo. PSUM
    evacuation alternates DVE/ACT so neither engine paces the chain.

Phase emission order keeps the TensorE stream dense: 96 identity
warmup matmuls bridge the DMA-bound head (keeping HAM/p-state at full
clock), the first attention block starts after only chunk 0's k/q
projections (the rest fed just-in-time as hT DMA blocks land on three
DGE queues), and projections for the other batch plus output
projection for finished blocks are popped as "filler" inside the
ScalarE-bound attention inner loop.

PSUM budget (8 banks): sc 2x[128,1024] (4) + ctx 2x[65,512] (2) +
pj 1x[128,512] (1) + po 1x[128,512] (1).
"""

import numpy as np
import ml_dtypes

import concourse.bass as bass
import concourse.mybir as mybir
import concourse.tile as tile
from concourse import bacc
from concourse import bass_utils

F32 = mybir.dt.float32
BF16 = mybir.dt.bfloat16
BF16_NP = ml_dtypes.bfloat16

B, S, D, H = 2, 2048, 1024, 16
HD = D // H
BS = B * S            # 4096
P = 128               # partitions / channels per core
NCORES = 8
KT = S // P           # 16 key tiles per batch
NQ = 512              # matmul moving free dim
VA_W = HD + 1         # v_aug columns per key tile (64 v cols + ones col)
QB = 512              # attention query block (ctx PSUM bank width)
NB = S // QB          # 4 query blocks per batch

_CACHE = {}


def _build():
    nc = bacc.Bacc("TRN2", target_bir_lowering=False, debug=False,
                   num_devices=NCORES)

    # all host-side pre-arranged for single contiguous DMAs
    hT = nc.dram_tensor("hT", [P, (D // P) * BS], BF16, kind="ExternalInput")
    wq = nc.dram_tensor("wq", [P, D], BF16, kind="ExternalInput")
    wk = nc.dram_tensor("wk", [P, D], BF16, kind="ExternalInput")
    wv = nc.dram_tensor("wv", [P, D], BF16, kind="ExternalInput")
    wo = nc.dram_tensor("wo", [P, D], BF16, kind="ExternalInput")
    bq = nc.dram_tensor("bq", [P, 1], F32, kind="ExternalInput")
    bk = nc.dram_tensor("bk", [P, 1], F32, kind="ExternalInput")
    bv = nc.dram_tensor("bv", [P, 1], F32, kind="ExternalInput")
    maskP = nc.dram_tensor("maskP", [P, B * KT], F32, kind="ExternalInput")
    sel = nc.dram_tensor("sel", [HD + 1, P], BF16, kind="ExternalInput")
    idn = nc.dram_tensor("idn", [P, P], BF16, kind="ExternalInput")
    outT = nc.dram_tensor("outT", [D, BS], BF16, kind="ExternalOutput")

    with tile.TileContext(nc) as tc:
        with (
            tc.tile_pool(name="const", bufs=1) as const,
            tc.tile_pool(name="res", bufs=1) as res,
            tc.tile_pool(name="va", bufs=2) as va_pool,
            tc.tile_pool(name="pr", bufs=3) as pr_pool,
            tc.tile_pool(name="bc", bufs=2) as bc_pool,
            tc.tile_pool(name="ot", bufs=4) as ot_pool,
            tc.tile_pool(name="pj_ps", bufs=1, space="PSUM") as pj_ps,
            tc.tile_pool(name="po_ps", bufs=1, space="PSUM") as po_ps,
            tc.tile_pool(name="sc_ps", bufs=2, space="PSUM") as sc_ps,
            tc.tile_pool(name="ctx_ps", bufs=2, space="PSUM") as ctx_ps,
        ):
            # ---- constants / weights in SBUF (contiguous DMAs) ----
            # The 8.75MB input load is HBM-bound per queue (~165GB/s
            # observed), so spread it over all three DGE queues (SP +
            # Activation HWDGE, GpSimd SWDGE). hT block k is needed at
            # roughly (7 + 3k)us; weights gate the very first matmul so
            # they lead the scalar queue.
            # one SBUF tile PER hT block: Tile's dependency tracking is
            # conservative per-tile (any read waits ALL writes), so a
            # single big tile pinned the first projection to the LAST
            # block's DMA (~35us) instead of its own block's (~17us)
            BLK = (D // P) * NQ   # 4096 cols per column block
            ht_blks = [const.tile([P, BLK], BF16, name=f"htb{n}",
                                  tag=f"htb{n}") for n in range(B * NB)]
            w_sbs = {}
            b_sbs = {}

            def _hblk(eng, n):
                eng.dma_start(ht_blks[n][:], hT.ap()[:, n * BLK:(n + 1) * BLK])

            for nm, w in (("wk", wk), ("wq", wq), ("wv", wv)):
                t = const.tile([P, D], BF16, name=f"{nm}_sb", tag=f"{nm}_sb")
                nc.scalar.dma_start(t[:], w.ap())
                w_sbs[nm] = t
            for nm, bt in (("bq", bq), ("bk", bk), ("bv", bv)):
                t = const.tile([P, 1], F32, name=f"{nm}_sb", tag=f"{nm}_sb")
                nc.scalar.dma_start(t[:], bt.ap())
                b_sbs[nm] = t
            identF = const.tile([P, P], BF16)
            nc.sync.dma_start(identF[:], idn.ap())
            _hblk(nc.sync, 0)
            mask_sb = const.tile([P, B * KT], F32)
            nc.gpsimd.dma_start(mask_sb[:], maskP.ap())
            # batch-0 blocks lead the two HWDGE queues (sync/scalar);
            # the gpsimd SWDGE queue barely moves before ~28us, so it only
            # carries b1 blocks that attention needs late anyway.
            # DMA engines round-robin descriptors across a queue's
            # pending entries, so a block completes near its queue's total
            # drain — keep the attention-gating b0 blocks on queues with
            # minimal co-pending bytes.
            _hblk(nc.sync, 2)
            _hblk(nc.scalar, 1)
            _hblk(nc.scalar, 3)
            _hblk(nc.gpsimd, 4)
            _hblk(nc.gpsimd, 5)
            _hblk(nc.gpsimd, 6)
            _hblk(nc.gpsimd, 7)
            # PE warmup: the input load is DMA-bound for ~30us and an idle
            # PE sits in a low HAM/p-state gear, making the first real
            # matmuls 2-3x slow. Identity matmuls (no DMA deps) keep the PE
            # busy from ~6.5us so it is at full clock when data lands.
            warm_ps = po_ps.tile([P, P], F32, name="warm", tag="po")
            for _ in range(96):
                nc.tensor.matmul(warm_ps[:], identF[:], identF[:],
                                 start=True, stop=True)

            qT = res.tile([P, BS], BF16)
            kT = res.tile([P, BS], BF16)
            vT = res.tile([P, BS], BF16)
            ctxraw = res.tile([P, BS], F32)
            ctxn = res.tile([P, BS], BF16)
            # softmax sums: the ctx evacuation writes h0's ones-row to
            # partition 0 and h1's to partition 64 (DVE cross-partition
            # copies are only legal at multiple-of-64 offsets), so the sel
            # matmul consumes them with no relocation step. Rows 1-63 are
            # memset once and zeroed by sel's 0 coefficients.
            s2_sb = res.tile([HD + 1, BS], BF16)
            nc.vector.memset(s2_sb[:], 0.0)

            VA = {}

            def setup_va(b):
                vas = []
                for h in range(2):
                    va = va_pool.tile([P, KT * VA_W], BF16, name=f"va{b}{h}",
                                      tag=f"va{h}")
                    nc.vector.memset(va[:], 1.0)
                    vas.append(va)
                VA[b] = vas

            def one_proj(wn, bn, dest, n):
                """one projection for one 512-col chunk, yielding after
                every contraction matmul (~0.4us PE each) so filler pops
                stay inside the attention loop's per-kt PE slack."""
                ps = pj_ps.tile([P, NQ], F32, name=f"ps_{wn}", tag="pj")
                for k in range(D // P):
                    nc.tensor.matmul(
                        ps[:], w_sbs[wn][:, bass.ts(k, P)],
                        ht_blks[n][:, k * NQ:(k + 1) * NQ],
                        start=(k == 0), stop=(k == D // P - 1))
                    if k % 2 == 1:
                        yield
                # high_priority: this evac frees the single pj bank; a lazy
                # DVE turnaround here stalls every later filler matmul
                with tc.high_priority():
                    nc.vector.tensor_scalar_add(
                        dest[:, bass.ts(n, NQ)], ps[:], b_sbs[bn][:])
                yield

            def tr_steps(b, ktlo, kthi):
                """v transposes for key tiles [ktlo,kthi): one [128,128] PE
                transpose covers BOTH heads' v slices; alternate pj/po
                banks so the DVE evacuation never head-of-line-blocks."""
                vas = VA[b]
                boff = b * S
                for kt in range(ktlo, kthi):
                    pool = pj_ps if kt % 2 == 0 else po_ps
                    tp = pool.tile([P, P], BF16, name="tp",
                                   tag="pj" if kt % 2 == 0 else "po")
                    nc.tensor.transpose(
                        tp[:], vT[:, boff + kt * P:boff + (kt + 1) * P],
                        identF[:])
                    nc.vector.tensor_copy(
                        vas[0][:, kt * VA_W:kt * VA_W + HD], tp[:, 0:HD])
                    nc.vector.tensor_copy(
                        vas[1][:, kt * VA_W:kt * VA_W + HD], tp[:, HD:P])
                    yield

            def proj_va_steps(b, nlo, nhi, with_q=True):
                """K/V projections + v_aug build for column chunks
                [nlo,nhi) of batch b (PE filler inside attention). k first:
                attention QKs gate on kT."""
                for n in range(b * NB + nlo, b * NB + nhi):
                    yield from one_proj("wk", "bk", kT, n)
                    yield from one_proj("wv", "bv", vT, n)
                    if with_q:
                        yield from one_proj("wq", "bq", qT, n)
                    nlocal = n - b * NB
                    yield from tr_steps(b, nlocal * 4, nlocal * 4 + 4)

            def chain(*gens):
                for g in gens:
                    yield from g

            def qk_pair(b, qb, kt):
                boff = b * S
                qsl = slice(boff + qb * QB, boff + (qb + 1) * QB)
                ksl = slice(boff + kt * P, boff + (kt + 1) * P)
                sct = sc_ps.tile([P, 2 * QB], F32, name="sct", tag="sct")
                nc.tensor.matmul(sct[:, 0:QB], kT[0:HD, ksl],
                                 qT[0:HD, qsl], start=True, stop=True)
                nc.tensor.matmul(sct[:, QB:2 * QB], kT[HD:P, ksl],
                                 qT[HD:P, qsl], start=True, stop=True)
                return sct

            def attn_qb(b, qb, filler, pops=1, quiet_head=0, quiet_tail=0,
                        sct0=None, next_blk=None):
                """Both heads' attention for one 512-query block. Each key
                tile: two concurrent row-group QK matmuls into one packed
                [128,1024] score tile, one exp for both heads, two PV
                accumulations. Pops filler steps to keep the PE dense."""
                va0, va1 = VA[b]
                boff = b * S
                qsl = slice(boff + qb * QB, boff + (qb + 1) * QB)
                ctx0 = ctx_ps.tile([VA_W, QB], F32, name="ctx0", tag="ctx")
                ctx1 = ctx_ps.tile([VA_W, QB], F32, name="ctx1", tag="ctx")



                # QK runs one key tile AHEAD of PV: the per-kt PE stream is
                # [QK(kt+1), pops, PV(kt)], so exp(kt+1) never waits on the
                # exp(kt)->PV(kt)->QK(kt+1) semaphore chain — the ACT
                # stream stays saturated at its 1.11us/exp floor.
                sct = sct0 if sct0 is not None else qk_pair(b, qb, 0)
                nxt = None
                for kt in range(KT):
                    pr = pr_pool.tile([P, 2 * QB], BF16, name="pr", tag="pr")
                    nc.scalar.activation(
                        pr[:], sct[:], mybir.ActivationFunctionType.Exp,
                        bias=mask_sb[:, b * KT + kt:b * KT + kt + 1],
                        scale=0.125)
                    if kt + 1 < KT:
                        sct = qk_pair(b, qb, kt + 1)
                    elif next_blk is not None:
                        nxt = qk_pair(next_blk[0], next_blk[1], 0)
                    # pops sit BETWEEN the QK pair and PV: a DMA- or
                    # DVE-gated filler here overlaps the exp wait instead of
                    # head-of-line-blocking the next block's QKs
                    if filler is not None and \
                            quiet_head <= kt < KT - quiet_tail:
                        for _ in range(pops):
                            next(filler, None)
                    nc.tensor.matmul(
                        ctx0[:], va0[:, kt * VA_W:(kt + 1) * VA_W],
                        pr[:, 0:QB], start=(kt == 0), stop=(kt == KT - 1))
                    nc.tensor.matmul(
                        ctx1[:], va1[:, kt * VA_W:(kt + 1) * VA_W],
                        pr[:, QB:2 * QB], start=(kt == 0),
                        stop=(kt == KT - 1))
                # fast evacuation: plain DVE copies release the ctx PSUM
                # slots; reciprocal happens later off-PSUM. high_priority so
                # the next block's PV reuse isn't blocked on a lazy DVE.
                with tc.high_priority():
                    nc.vector.tensor_copy(ctxraw[0:HD, qsl], ctx0[0:HD, :])
                    nc.vector.tensor_copy(s2_sb[0:1, qsl],
                                          ctx0[HD:HD + 1, :])
                    nc.vector.tensor_copy(ctxraw[HD:P, qsl], ctx1[0:HD, :])
                    nc.vector.tensor_copy(s2_sb[HD:HD + 1, qsl],
                                          ctx1[HD:HD + 1, :])
                return nxt

            def gather_norm(b, qb, use_dve=False):
                """normalize ctxT for one 512-column block: broadcast the
                two heads' sums (at partitions 0/64) via sel matmul, recip,
                multiply. high_priority so the chain threads in. The
                multiply runs on GpSimd (idle engine, SBUF-only operands)
                to keep the DVE free — except use_dve for the final block,
                where DVE's lower dispatch latency shortens the tail."""
                with tc.high_priority():
                    goff = b * S + qb * QB
                    pbc = po_ps.tile([P, QB], F32, name="pbc", tag="po")
                    nc.tensor.matmul(pbc[:], sel_sb[:],
                                     s2_sb[:, goff:goff + QB],
                                     start=True, stop=True)
                    bcr = bc_pool.tile([P, QB], F32, name="bcr", tag="bcr")
                    nc.vector.reciprocal_approx_fast(bcr[:], pbc[:])
                    eng = nc.vector if use_dve else nc.gpsimd
                    eng.tensor_mul(
                        ctxn[:, goff:goff + QB],
                        ctxraw[:, goff:goff + QB], bcr[:])

            def gn_gen(b, qb):
                """gather_norm as a single-pop filler: emitted inside the
                NEXT block's kt loop, so its sel matmul never sits in the
                PE stream at a block boundary waiting on the sums evac."""
                gather_norm(b, qb)
                yield

            def oproj_steps(b, blo, bhi, tail=False):
                """partial output projection for 512-col blocks [blo,bhi) of
                batch b: outT[o, n] += Wo[o, own chans] @ ctxn — full o
                range, own 128 channels; cross-core reduction on host.
                PSUM evacuation alternates DVE/ACT: a single engine paces
                the whole chain at ~0.9us per matmul (bank round-trip) and
                that crawl head-of-line-blocks the attention QKs behind it.
                tail mode (attention done, ctx banks free) rotates 4 PSUM
                banks and 2 DMA queues to minimize the drain latency."""
                boff = b * S
                pools = ([po_ps, pj_ps, ctx_ps, ctx_ps] if tail
                         else [po_ps, pj_ps])
                tags = ["po", "pj", "ctx", "ctx"]
                for cg in range(blo, bhi):
                    goff = boff + cg * QB
                    for t in range(D // P):
                        m = t % len(pools)
                        po = pools[m].tile([P, QB], F32, name="po",
                                           tag=tags[m])
                        nc.tensor.matmul(
                            po[:], wo_sb[:, bass.ts(t, P)],
                            ctxn[:, goff:goff + QB],
                            start=True, stop=True)
                        ot = ot_pool.tile([P, QB], BF16, name="ot", tag="ot")
                        if t % 2 == 1:
                            nc.scalar.activation(
                                ot[:], po[:],
                                mybir.ActivationFunctionType.Copy, bias=0.0)
                        else:
                            nc.vector.tensor_copy(ot[:], po[:])
                        dq = nc.scalar if (tail and t % 2 == 1) else nc.sync
                        dq.dma_start(
                            outT.ap()[bass.ts(t, P), goff:goff + QB], ot[:])
                        if t % 2 == 1:
                            yield
                    yield

            class FQ:
                """Filler queue: generators become poppable only once
                pushed, so a filler that reads a region (e.g. o-proj on
                ctxn) is never EMITTED before its producer (gather_norm)
                — Tile deps are emission-order-based."""

                def __init__(self):
                    self.gens = []

                def push(self, g):
                    self.gens.append(g)

                def push_front(self, g):
                    self.gens.insert(0, g)

                def __next__(self):
                    while self.gens:
                        try:
                            return next(self.gens[0])
                        except StopIteration:
                            self.gens.pop(0)
                    return None

            def drain(g):
                if isinstance(g, FQ):
                    while g.gens:
                        next(g)
                    return
                for _ in g:
                    pass

            # software pipeline: engines run their streams in-order, so
            # anything that waits on a slow dependency must sit at a stream
            # position where that dependency is already resolved.
            setup_va(0)
            # only chunk 0's k and q projected serially; v+transposes and
            # chunks 1-3 are filler inside the first attention block
            # (QK(kt) gates on kT chunk kt//4, PV(kt) on va(kt) — produced
            # just in time as the DMA blocks land). Attention's exp stream
            # starts as soon as the first hT block is projected.
            drain(chain(one_proj("wk", "bk", kT, 0),
                        one_proj("wq", "bq", qT, 0)))
            # deferred constant loads: not needed until gather_norm/o-proj,
            # so keep them out of the startup DMA burst
            wo_sb = const.tile([P, D], BF16)
            nc.sync.dma_start(wo_sb[:], wo.ap())
            sel_sb = const.tile([HD + 1, P], BF16)
            nc.sync.dma_start(sel_sb[:], sel.ap())
            # pops=6: fA0's yield c for PV(kt)/QK(kt) emission deadlines —
            # v-ch0 y1-5, tr kt0-3 y6-9, chunk c at y10+19(c-1): PV(kt)
            # sees (kt+1)*p pops, QK(kt) sees kt*p; p=6 satisfies all
            # (tightest: PV(0) needs y6 <= 6).
            fA0 = chain(one_proj("wv", "bv", vT, 0), tr_steps(0, 0, 4),
                        proj_va_steps(0, 1, 4))
            s = attn_qb(0, 0, fA0, pops=6, next_blk=(0, 1))
            drain(fA0)
            setup_va(1)
            # b1 k/v proj as filler in A(b0); ALL of b1's q proj is
            # deferred into A(b1) to balance the two windows' PE load.
            # quiet_head on qb1: b1's hT blocks are still in flight; a
            # DMA-gated filler matmul would head-of-line-block the QKs
            # behind it in the PE stream.
            fq0 = FQ()
            fq0.push(gn_gen(0, 0))
            fq0.push(proj_va_steps(1, 0, 4, with_q=False))
            s = attn_qb(0, 1, fq0, pops=2, quiet_head=6, sct0=s,
                        next_blk=(0, 2))
            fq0.push_front(gn_gen(0, 1))
            s = attn_qb(0, 2, fq0, pops=2, sct0=s, next_blk=(0, 3))
            fq0.push_front(gn_gen(0, 2))
            # q0 (b1 chunk-0 q proj) is popped inside attn(0,3) so the
            # cross-block QK(1,0,kt0) emitted at its tail has its input
            fq0.push(one_proj("wq", "bq", qT, NB))
            s = attn_qb(0, 3, fq0, pops=2, quiet_tail=2, sct0=s,
                        next_blk=(1, 0))
            fq0.push_front(gn_gen(0, 3))
            drain(fq0)                          # b1 k/v/q0 + b0 norms done
            fq = FQ()                           # A(b1) fillers
            q1 = one_proj("wq", "bq", qT, NB + 1)
            fq.push(q1)
            fq.push(oproj_steps(0, 0, NB))
            s = attn_qb(1, 0, fq, pops=2, quiet_head=2, quiet_tail=3,
                        sct0=s, next_blk=(1, 1))
            drain(q1)                           # qT chunk 1 for attn(1,1)
            fq.push_front(gn_gen(1, 0))
            fq.push(oproj_steps(1, 0, 1))
            q2 = one_proj("wq", "bq", qT, NB + 2)
            fq.push(q2)
            s = attn_qb(1, 1, fq, pops=2, quiet_tail=3, sct0=s,
                        next_blk=(1, 2))
            drain(q2)
            fq.push_front(gn_gen(1, 1))
            fq.push(oproj_steps(1, 1, 2))
            q3 = one_proj("wq", "bq", qT, NB + 3)
            fq.push(q3)
            s = attn_qb(1, 2, fq, pops=2, quiet_tail=3, sct0=s,
                        next_blk=(1, 3))
            drain(q3)
            fq.push_front(gn_gen(1, 2))
            fq.push(oproj_steps(1, 2, 3))
            attn_qb(1, 3, fq, pops=2, quiet_tail=3, sct0=s)
            drain(fq)
            gather_norm(1, 3, use_dve=True)
            drain(oproj_steps(1, NB - 1, NB, tail=True))

    nc.compile()
    return nc


def _prep_inputs(hidden_state, attention_mask, Wq, bq, Wk, bk, Wv, bv, Wo,
                 bo):
    # hT blocks: hTr[p, (n*8+k)*512 + m] = h2[k*128+p, n*512+m]
    h2 = np.ascontiguousarray(
        np.asarray(hidden_state, dtype=np.float32).reshape(BS, D).T)
    h3 = h2.reshape(D // P, P, B * NB, NQ)          # [k, p, n, m]
    hTr = np.ascontiguousarray(
        h3.transpose(1, 2, 0, 3).reshape(P, (D // P) * BS)).astype(BF16_NP)
    # mask: maskP[p, b*KT + t] = mask[b, t*128+p]
    m2 = np.asarray(attention_mask, dtype=np.float32).reshape(B, S)
    maskP = np.ascontiguousarray(
        m2.reshape(B, KT, P).transpose(2, 0, 1).reshape(P, B * KT))
    idnm = np.eye(P, dtype=np.float32).astype(BF16_NP)
    selm = np.zeros((HD + 1, P), dtype=BF16_NP)
    selm[0, 0:HD] = 1
    selm[HD, HD:P] = 1

    def warr(Wslice):
        # w[p, k*128 + c] = Wslice.T[k*128+p, c]
        wt = np.asarray(Wslice, dtype=np.float32).T     # [D, P]
        return np.ascontiguousarray(
            wt.reshape(D // P, P, P).transpose(1, 0, 2).reshape(P, D)
        ).astype(BF16_NP)

    in_maps = []
    for c in range(NCORES):
        sl = slice(c * P, (c + 1) * P)
        in_maps.append({
            "hT": hTr,
            "wq": warr(np.asarray(Wq)[sl, :]),
            "wk": warr(np.asarray(Wk)[sl, :]),
            "wv": warr(np.asarray(Wv)[sl, :]),
            "wo": np.ascontiguousarray(
                np.asarray(Wo, dtype=np.float32)[:, sl].T).astype(BF16_NP),
            "bq": np.asarray(bq, dtype=np.float32)[sl].reshape(P, 1),
            "bk": np.asarray(bk, dtype=np.float32)[sl].reshape(P, 1),
            "bv": np.asarray(bv, dtype=np.float32)[sl].reshape(P, 1),
            "maskP": maskP,
            "sel": selm,
            "idn": idnm,
        })
    return in_maps


def kernel(**inputs) -> np.ndarray:
    if "nc" not in _CACHE:
        _CACHE["nc"] = _build()
    nc = _CACHE["nc"]
    in_maps = _prep_inputs(**inputs)
    res = bass_utils.run_bass_kernel_spmd(
        nc, in_maps, core_ids=list(range(NCORES)))
    outT = res.results[0]["outT"].astype(np.float32)  # [D, BS] partials
    for c in range(1, NCORES):
        outT += res.results[c]["outT"].astype(np.float32)
    out = np.ascontiguousarray(outT.T).reshape(B, S, D)
    out += np.asarray(inputs["bo"], dtype=np.float32)
    return out.astype(np.float32)
